# revision 9
# baseline (speedup 1.0000x reference)
"""DAGLayer (gnn_message_passing) Trainium2 kernel, 8-core data-parallel.

Strategy (v2):
- Shard 6400 rows across 8 cores (800 rows/core, as 2 halves of 400 rows on
  partition ranges 0:64 / 64:128).
- Device keeps an append-only *output history* Y[128, col] in SBUF; all data
  dependence lives in host-precomputed gather indices (last-writer simulation
  done in numpy at kernel-call time; the bass program is built against the
  per-step compacted-count schedule derived from the masks).
- Mask compaction: per step only rows whose calculation_mask is set (~60%) get
  a column; counts padded to >=256 (fp32r fast-path) and a multiple of 16.
- Per step: ONE gpsimd ap_gather builds the full parent-feature tile
  [128, 49*n] in matmul layout (one instruction amortizes the ~7us GPSIMD
  ucode launch overhead that dominated v1); 98 fp32r matmuls (K=64, 1 cyc/row)
  accumulate layer 1 into two PSUM banks; the atom-feature contribution
  af@W1a+b1 is host-precomputed, streamed, added on DVE; relu; fp32r layer 2;
  relu+b2 appended to Y (or DMA'd out on the final step).
"""
import sys, time
sys.path.insert(0, '/opt/trn_rl_repo')
import numpy as np
from concourse.alu_op_type import AluOpType

N_TOTAL = 6400
A = 50              # max_atoms / steps
G = 64              # graph feat
H = 128             # hidden
NAF = 75            # atom feat
NJ = A - 1          # 49 parent positions
NCORES = 8
R = N_TOTAL // NCORES      # 800 rows per core
RH = R // 2                # 400 rows per half

_cache = {}


def _build(nm, yoff, ioff, aoff, YC):
    """nm[t]: padded masked count per step; yoff[t]: Y col offset (steps 0..48
    stored); ioff[t]: idx flat col offset; aoff[t]: afw flat col offset;
    YC: total Y history cols (zero col index = YC)."""
    import concourse.bass as bass
    import concourse.mybir as mybir
    import concourse.tile as tile
    from concourse import bacc, library_config

    DT = mybir.dt.float32
    FR = mybir.dt.float32r
    I16 = mybir.dt.int16
    I32 = mybir.dt.int32
    YCp1 = YC + 1
    MAXN = max(nm)
    tot_icols = ioff[-1]
    tot_acols = aoff[-1]

    nc = bacc.Bacc("TRN2", target_bir_lowering=False, debug=False, num_devices=NCORES)
    afw_dram = nc.dram_tensor("afw", [128, tot_acols], DT, kind="ExternalInput")
    idx_dram = nc.dram_tensor("idx", [128, tot_icols], I16, kind="ExternalInput")
    w1_dram = nc.dram_tensor("w1d", [128, NJ * H], FR, kind="ExternalInput")
    w2_dram = nc.dram_tensor("w2", [H, G], FR, kind="ExternalInput")
    b2_dram = nc.dram_tensor("b2c", [G, 1], DT, kind="ExternalInput")
    o_dram = nc.dram_tensor("out", [G, R], DT, kind="ExternalOutput")

    with tile.TileContext(nc) as tc:
        with (
            tc.tile_pool(name="state", bufs=1) as state,
            tc.tile_pool(name="stream", bufs=3) as stream,
            tc.tile_pool(name="gt", bufs=1) as gtp,
            tc.tile_pool(name="hid", bufs=2) as hidp,
            tc.tile_pool(name="ps1", bufs=2, space="PSUM") as ps1p,
            tc.tile_pool(name="ps2", bufs=2, space="PSUM") as ps2p,
        ):
            ysb = state.tile([128, YCp1], FR)
            w1sb = state.tile([128, NJ * H], FR)
            w2sb = state.tile([H, G], FR)
            b2sb = state.tile([G, 1], DT)
            nc.sync.dma_start(w1sb[:, :], w1_dram[:, :])
            nc.sync.dma_start(w2sb[:, :], w2_dram[:, :])
            nc.sync.dma_start(b2sb[:, :], b2_dram[:, :])
            nc.vector.memset(ysb[:, :].bitcast(DT), 0.0)
            nc.gpsimd.load_library(library_config.ap_gather)

            import os as _os
            for t in range(int(_os.environ.get('TSTEPS', A))):
                n = nm[t]
                m = NJ * n
                afw = stream.tile([H, 2 * MAXN], DT, tag="afw")
                nc.sync.dma_start(afw[:, 0:2 * n], afw_dram[:, aoff[t]:aoff[t] + 2 * n])
                if t > 0:
                    idxt = stream.tile([128, NJ * MAXN // 16], I16, tag="idx")
                    nc.sync.dma_start(idxt[:, 0:m // 16],
                                      idx_dram[:, ioff[t]:ioff[t] + m // 16])
                    gt = gtp.tile([128, NJ * MAXN], FR, tag="gt")
                    nc.gpsimd.ap_gather(
                        gt[:, 0:m], ysb[:, :], idxt[:, 0:m // 16],
                        channels=128, num_elems=YCp1, d=1, num_idxs=m,
                    )

                hidA = hidp.tile([H, MAXN], FR, tag="hidA")
                hidB = hidp.tile([H, MAXN], FR, tag="hidB")
                if t > 0:
                    psA = ps1p.tile([128, MAXN], DT, tag="psA")
                    psB = ps1p.tile([128, MAXN], DT, tag="psB")
                    for j in range(NJ):
                        nc.tensor.matmul(
                            psA[:, 0:n],
                            w1sb[0:64, j * H:(j + 1) * H],
                            gt[0:64, j * n:(j + 1) * n],
                            start=(j == 0), stop=(j == NJ - 1))
                        nc.tensor.matmul(
                            psB[:, 0:n],
                            w1sb[64:128, j * H:(j + 1) * H],
                            gt[64:128, j * n:(j + 1) * n],
                            start=(j == 0), stop=(j == NJ - 1))
                    nc.vector.tensor_tensor(hidA[:, 0:n], psA[:, 0:n], afw[:, 0:n], AluOpType.add)
                    nc.vector.tensor_tensor(hidB[:, 0:n], psB[:, 0:n], afw[:, n:2 * n], AluOpType.add)
                    nc.vector.tensor_scalar_max(hidA[:, 0:n], hidA[:, 0:n], 0.0)
                    nc.vector.tensor_scalar_max(hidB[:, 0:n], hidB[:, 0:n], 0.0)
                else:
                    # step 0 has no written history: layer-1 parent term is zero
                    nc.vector.tensor_scalar_max(hidA[:, 0:n], afw[:, 0:n], 0.0)
                    nc.vector.tensor_scalar_max(hidB[:, 0:n], afw[:, n:2 * n], 0.0)

                ps2A = ps2p.tile([G, MAXN], DT, tag="ps2A")
                ps2B = ps2p.tile([G, MAXN], DT, tag="ps2B")
                nc.tensor.matmul(ps2A[:, 0:n], w2sb[:, :],
                                 hidA[:, 0:n], start=True, stop=True)
                nc.tensor.matmul(ps2B[:, 0:n], w2sb[:, :],
                                 hidB[:, 0:n], start=True, stop=True)

                if t == A - 1:
                    outT = state.tile([G, R], DT)
                    nc.scalar.activation(outT[:, 0:RH], ps2A[:, 0:RH],
                                         mybir.ActivationFunctionType.Relu, bias=b2sb[:, :])
                    nc.scalar.activation(outT[:, RH:R], ps2B[:, 0:RH],
                                         mybir.ActivationFunctionType.Relu, bias=b2sb[:, :])
                    nc.sync.dma_start(o_dram[:, :], outT[:, :])
                else:
                    nc.scalar.activation(ysb[0:64, yoff[t]:yoff[t] + n], ps2A[:, 0:n],
                                         mybir.ActivationFunctionType.Relu, bias=b2sb[:, :])
                    nc.scalar.activation(ysb[64:128, yoff[t]:yoff[t] + n], ps2B[:, 0:n],
                                         mybir.ActivationFunctionType.Relu, bias=b2sb[:, :])

    nc.compile()
    _patch_gather_dtype(nc)
    return nc


def _patch_gather_dtype(nc):
    """The ap_gather Q7 ucode mishandles the float32r val_dtype encoding (11)
    and hangs on hardware. The gather is a pure bit-mover, so patch the
    encoded val_dtype to float32 (10) in the serialized BIR while keeping the
    BIR-level AP dtypes float32r (which the walrus verifier requires for data
    consumed by fp32r matmuls)."""
    import json as _json
    orig = nc.to_json_bytes

    def patched():
        j = _json.loads(orig())
        nfix = 0

        def walk(o):
            nonlocal nfix
            if isinstance(o, dict):
                if o.get("op_name") == "APGather" and "instr" in o:
                    if o["instr"][14] == 11:
                        o["instr"][14] = 10
                        nfix += 1
                for v in o.values():
                    walk(v)
            elif isinstance(o, list):
                for v in o:
                    walk(v)

        walk(j)
        return _json.dumps(j).encode()

    nc.to_json_bytes = patched


def _compiled_runner(nc):
    import jax
    from jax.sharding import Mesh, PartitionSpec, NamedSharding
    from jax.experimental.shard_map import shard_map
    import concourse.mybir as mybir
    from concourse.bass2jax import _bass_exec_p, partition_id_tensor, install_neuronx_cc_hook

    install_neuronx_cc_hook()
    partition_name = nc.partition_id_tensor.name if nc.partition_id_tensor else None
    in_names, out_names, out_avals, zero_outs = [], [], [], []
    for alloc in nc.m.functions[0].allocations:
        if not isinstance(alloc, mybir.MemoryLocationSet):
            continue
        name = alloc.memorylocations[0].name
        if alloc.kind == "ExternalInput":
            if name != partition_name:
                in_names.append(name)
        elif alloc.kind == "ExternalOutput":
            shape = tuple(alloc.tensor_shape)
            dtype = mybir.dt.np(alloc.dtype)
            out_names.append(name)
            out_avals.append(jax.core.ShapedArray(shape, dtype))
            zero_outs.append(np.zeros(shape, dtype))
    all_in = in_names + out_names + ([partition_name] if partition_name else [])

    def _body(*args):
        operands = list(args)
        if partition_name is not None:
            operands.append(partition_id_tensor())
        return tuple(_bass_exec_p.bind(
            *operands, out_avals=tuple(out_avals), in_names=tuple(all_in),
            out_names=tuple(out_names), lowering_input_output_aliases=(),
            sim_require_finite=False, sim_require_nnan=False, nc=nc))

    devices = jax.devices()[:NCORES]
    mesh = Mesh(np.asarray(devices), ("core",))
    n_params, n_outs = len(in_names), len(out_names)
    fn = jax.jit(shard_map(_body, mesh=mesh,
                           in_specs=(PartitionSpec("core"),) * (n_params + n_outs),
                           out_specs=(PartitionSpec("core"),) * n_outs, check_rep=False),
                 keep_unused=True)
    return fn, in_names, out_names, out_avals, zero_outs, mesh


def _precompute(atom_features, W1, b1, W2, b2, parents, calculation_orders, calculation_masks):
    par = np.asarray(parents).astype(np.int64)
    orders = np.asarray(calculation_orders).astype(np.int64)
    masks = np.asarray(calculation_masks).astype(bool).copy()
    masks[:, A - 1] = True
    af = np.asarray(atom_features, dtype=np.float32)
    W1 = np.asarray(W1, dtype=np.float32)

    # per-(core,half,step) masked local-row lists; padded count nm[t]
    mh = masks.reshape(NCORES, 2, RH, A)          # [core, half, rl, t]
    cnt = mh.sum(axis=2)                           # [core, half, t]
    nm_raw = cnt.max(axis=(0, 1))                  # [t]
    # >=256 keeps fp32r matmuls on the 1-cycle/row path; x16 keeps the
    # wrapped idx blocks aligned
    nm = np.maximum(256, (nm_raw + 15) // 16 * 16).astype(np.int64)
    yoff = np.concatenate([[0], np.cumsum(nm[:A - 1])]).astype(np.int64)
    YC = int(yoff[-1])                             # zero col index
    ioff = np.concatenate([[0], np.cumsum(NJ * (nm // 16))]).astype(np.int64)
    aoff = np.concatenate([[0], np.cumsum(2 * nm)]).astype(np.int64)

    # position of each (row, t) in its compacted list (or -1)
    pos = -np.ones((N_TOTAL, A), np.int64)
    lists = {}                                     # (core, half, t) -> local row array
    for c in range(NCORES):
        for h in range(2):
            for t in range(A):
                rl = np.nonzero(mh[c, h, :, t])[0]
                lists[(c, h, t)] = rl
                gr = c * R + h * RH + rl
                pos[gr, t] = np.arange(rl.size)

    # last-writer simulation -> (src step, src pos) per (row, step, parent j)
    lastw_t = np.full((N_TOTAL, A), -1, np.int64)
    src_t = np.empty((N_TOTAL, A, NJ), np.int64)
    rows = np.arange(N_TOTAL)
    for t in range(A):
        src_t[:, t, :] = np.take_along_axis(lastw_t, par[:, t, 1:], axis=1)
        m = masks[:, t]
        lastw_t[rows[m], par[m, t, 0]] = t

    # idx value: yoff[src_t] + pos[row, src_t] if src >= 0 else YC
    pos_at_src = np.take_along_axis(pos, np.maximum(src_t, 0).reshape(N_TOTAL, -1), axis=1).reshape(N_TOTAL, A, NJ)
    idxval_full = np.where(src_t >= 0, yoff[np.maximum(src_t, 0)] + pos_at_src, YC).astype(np.int64)
    assert idxval_full.max() <= YC < 32768

    # af @ W1a + b1 once for all rows, then index  (cheap: [6400,75]@[75,128])
    afW_all = af @ W1[:NAF] + np.asarray(b1, np.float32)   # [N_TOTAL, H]

    idx_w = np.zeros((NCORES, 128, int(ioff[-1])), np.int16)
    afw_w = np.zeros((NCORES, 128, int(aoff[-1])), np.float32)
    for t in range(A):
        n = int(nm[t])
        w = NJ * n // 16
        for c in range(NCORES):
            for h in range(2):
                rl = lists[(c, h, t)]
                gr = c * R + h * RH + rl
                iv = np.full((n, NJ), YC, np.int64)
                iv[:rl.size, :] = idxval_full[gr, t, :]          # [n_real, NJ]
                # flat index i = j*n + col -> partition i%16, col i//16
                blk = iv.T.reshape(w, 16).T.astype(np.int16)     # [16, w]
                for g in range(4):
                    idx_w[c, 16 * (4 * h + g):16 * (4 * h + g) + 16,
                          int(ioff[t]):int(ioff[t]) + w] = blk
                av = np.zeros((n, H), np.float32)
                av[:rl.size, :] = afW_all[orders[gr, t]]
                afw_w[c, :, int(aoff[t]) + h * n:int(aoff[t]) + (h + 1) * n] = av.T

    w1pg = W1[NAF:].reshape(NJ, G, H)
    w1d = np.concatenate([w1pg, w1pg], axis=1)     # [NJ, 128, H]
    w1d = w1d.transpose(1, 0, 2).reshape(128, NJ * H).copy()
    w2 = np.asarray(W2, dtype=np.float32).copy()
    b2c = np.asarray(b2, dtype=np.float32).reshape(G, 1).copy()
    return (idx_w, afw_w, w1d, w2, b2c,
            nm.tolist(), yoff.tolist(), ioff.tolist(), aoff.tolist(), YC, lists)


def kernel(atom_features, W1, b1, W2, b2, parents, calculation_orders,
           calculation_masks, n_atoms=None, **_ignored):
    import jax
    from jax.sharding import PartitionSpec, NamedSharding

    (idx_w, afw_w, w1d, w2, b2c, nm, yoff, ioff, aoff, YC, _lists) = _precompute(
        atom_features, W1, b1, W2, b2, parents, calculation_orders, calculation_masks)

    if "nc" not in _cache:
        _cache["nc"] = _build(nm, yoff, ioff, aoff, YC)
        _cache["runner"] = _compiled_runner(_cache["nc"])
    fn, in_names, out_names, out_avals, zero_outs, mesh = _cache["runner"]

    per_core = {
        "afw": afw_w,
        "idx": idx_w,
        "w1d": np.broadcast_to(w1d, (NCORES, *w1d.shape)),
        "w2": np.broadcast_to(w2, (NCORES, *w2.shape)),
        "b2c": np.broadcast_to(b2c, (NCORES, *b2c.shape)),
    }
    concat_in = [np.ascontiguousarray(per_core[n].reshape(-1, *per_core[n].shape[2:]))
                 for n in in_names]
    concat_zeros = [np.zeros((NCORES * z.shape[0], *z.shape[1:]), z.dtype) for z in zero_outs]
    args = [jax.device_put(a, NamedSharding(mesh, PartitionSpec("core")))
            for a in [*concat_in, *concat_zeros]]
    out = fn(*args)
    jax.block_until_ready(out)
    times = []
    for _ in range(3):
        t0 = time.time()
        out = fn(*args)
        jax.block_until_ready(out)
        times.append(time.time() - t0)
    _cache["exec_wall_s"] = min(times)

    _cache["fn_args"] = (fn, args, out_names)
    o = np.asarray(out[out_names.index("out")]).reshape(NCORES, G, R)
    res = o.transpose(0, 2, 1).reshape(N_TOTAL, G).astype(np.float32)
    return res
